# revision 28
# baseline (speedup 1.0000x reference)
"""AlphatRNN Trainium2 kernel (8 NeuronCores, data-parallel over batch).

Problem: 2-layer alpha-RNN, B=64, T=2048, I=128, H=256.
  per step t, layer l:
    cand  = tanh(cur @ Wih + h_l @ Whh + bh)
    alpha = sigmoid(cur @ Wax + h_l @ Wah + ba)
    h_l   = alpha * cand + (1 - alpha) * h_l ;  cur = h_l
  outputs = h_1 over time [B, T, H];  h_final [2, B, H]

Design notes:
  - Batch sharded 8 ways (8 rows/core); both layers per core; T is serial.
  - Everything lives transposed on chip: h as [H partitions, (step, batch)].
  - sigmoid(z) = 0.5*tanh(z/2)+0.5 folded into weights => ONE tanh per
    layer-step over [128, 32] covering cand|alpha pre-activations.
  - x-projections computed in bulk per chunk of C steps directly into PSUM;
    per-step recurrent matmuls accumulate into the same PSUM regions
    (has_written bits stay set), so no separate add op is needed.
  - h state kept in bf16 (validated: rel_l2 ~5e-3 vs f32 reference).
  - blend uses fused scalar_tensor_tensor ops: 3 VectorE ops per layer-step.
  - Hardware allows ONE semaphore wait per instruction. Tile emits a wait per
    unobserved dep proc, including redundant same-engine ones. We (a) drop
    same-engine waits post-schedule (engines execute/complete in FIFO order),
    and (b) insert tiny "absorber" ops so each cross-engine tick is observed
    by an instruction that needs no other wait.
"""

import numpy as np

B, T_FULL, I, H = 64, 2048, 128, 256
NCORES = 8
B_LOC = B // NCORES  # 8


def _drop_self_waits(nc):
    """Remove waits on an engine's own semaphore: engines issue and complete
    instructions in order, so same-engine deps are satisfied by program order.
    Necessary because most instruction encodings have one sync-wait slot."""
    import bass_rust

    prefix = {
        "EngineType.PE": "PE_",
        "EngineType.DVE": "DVE_",
        "EngineType.Activation": "Activation_",
    }
    # Identify the ringchain absorbers (the last 8 DMACopies): verify they
    # cover all 8 rings and find the last one's ring semaphore.
    dmas = []
    for blk in nc.m.functions[0].blocks:
        for inst in blk.instructions:
            if inst.concise_opcode() == "DMACopy":
                dmas.append(inst)
    last_ring_sem = None
    if len(dmas) >= 8:
        tailsems = [
            d.sync_info.on_update[0].ant_name
            for d in dmas[-8:]
            if d.sync_info and d.sync_info.on_update
        ]
        if len(set(tailsems)) == 8:
            last_ring_sem = tailsems[-1]
    name2inst = {}
    engpos = {}
    engcnt = {}
    for blk in nc.m.functions[0].blocks:
        for inst in blk.instructions:
            name2inst[inst.name] = inst
            e = str(inst.engine)
            engpos[inst.name] = engcnt.get(e, 0)
            engcnt[e] = engcnt.get(e, 0) + 1
    observed = {}
    n_drop = 0
    for blk in nc.m.functions[0].blocks:
        for inst in blk.instructions:
            si = inst.sync_info
            if not si or not si.on_wait:
                continue
            if inst.concise_opcode() == "Drain":
                # Engine-stream completion is implied by the all-engine
                # barrier that follows; DMA-ring completion is implied by the
                # ringchain absorber chain: the last absorber's ring semaphore
                # transitively covers every ring (chain RAW waits + ring
                # FIFO). So the drain needs exactly that one wait.
                if last_ring_sem is not None and len(si.on_wait) > 1:
                    kept = [w for w in si.on_wait if w.ant_name == last_ring_sem]
                    assert len(kept) == 1, (
                        f"ringchain drain fix: expected 1 wait on "
                        f"{last_ring_sem}, got {[(w.ant_name, w.wait_value) for w in si.on_wait]}"
                    )
                    n_drop += len(si.on_wait) - len(kept)
                    inst.sync_info = bass_rust.SyncInfo(
                        on_wait=kept, on_update=list(si.on_update)
                    )
                continue
            if inst.concise_opcode() == "DMACopy":
                # A DMA ring executes its descriptors serially, so a wait on a
                # semaphore this DMA itself updates (= its own ring's sem) is
                # satisfied by ring order.
                own = {u.ant_name for u in si.on_update}
                kept = [w for w in si.on_wait if w.ant_name not in own]
                if len(kept) == 2:
                    # [engine-wait, DMAHW-wait] = [slot readers, slot's prior
                    # writer]. Readers waited on the prior write themselves,
                    # so the engine wait transitively covers the DMAHW one.
                    eng_w = [w for w in kept if not w.ant_name.startswith("DMAHW")]
                    dma_w = [w for w in kept if w.ant_name.startswith("DMAHW")]
                    if len(eng_w) == 1 and len(dma_w) == 1:
                        kept = eng_w
                if len(kept) != len(si.on_wait):
                    n_drop += len(si.on_wait) - len(kept)
                    inst.sync_info = bass_rust.SyncInfo(
                        on_wait=kept, on_update=list(si.on_update)
                    )
                continue
            if inst.concise_opcode() in (
                "EventSemaphore",
                "EVENT_SEMAPHORE_RANGE_CLEAR",
            ):
                continue
            eng = str(inst.engine)
            pref = prefix.get(eng)
            if pref is None:
                continue
            obs = observed.setdefault(eng, {})
            # Same-engine RAW dep? Then the self-sem wait is load-bearing:
            # the engine pipeline lets op N+1 read inputs before op N's
            # writes drain (measured ~100ns overlap on DVE), so only a
            # semaphore wait (sem incs at completion) makes the write
            # visible. WAR/WAW self deps are safe: reads happen in early
            # pipe stages, writes commit in order at the end.
            self_raw = False
            try:
                my_ins = {
                    a.memref for a in inst.ins if hasattr(a, "memref")
                }
            except Exception:
                my_ins = set()
            try:
                depnames = list(inst.sync_dependency_names())
            except Exception:
                depnames = []
            for dn in depnames:
                dep = name2inst.get(dn)
                if dep is None or str(dep.engine) != eng:
                    continue
                # A same-engine RAW hazard only bites within the engine's
                # pipeline depth (~1 op of overlap observed); a producer >=3
                # slots back has fully drained by issue order.
                if engpos[inst.name] - engpos[dn] >= 3:
                    continue
                try:
                    douts = {a.memref for a in dep.outs if hasattr(a, "memref")}
                except Exception:
                    douts = set()
                if douts & my_ins:
                    self_raw = True
                    break
            kept = []
            for w in si.on_wait:
                nm = w.ant_name or ""
                if obs.get(nm, -1) >= w.wait_value:
                    continue  # already observed by this engine: redundant
                if nm.startswith(pref) and not self_raw:
                    continue  # self-sem wait guarding only WAR/WAW: safe
                kept.append(w)
            assert len(kept) <= 1, (
                f"{inst.name} {inst.concise_opcode()} on {eng} still needs "
                f"{[(w.ant_name, w.wait_value) for w in kept]} "
                f"(self_raw={self_raw}) - add an absorber op"
            )
            for w in kept:
                obs[w.ant_name] = max(obs.get(w.ant_name, -1), w.wait_value)
            if len(kept) != len(si.on_wait):
                n_drop += len(si.on_wait) - len(kept)
                inst.sync_info = bass_rust.SyncInfo(
                    on_wait=kept, on_update=list(si.on_update)
                )
    return n_drop


def _max_waits(nc):
    worst = {}
    for blk in nc.m.functions[0].blocks:
        for inst in blk.instructions:
            si = inst.sync_info
            nw = len(si.on_wait) if si else 0
            op = inst.concise_opcode()
            if op in ("Drain", "EVENT_SEMAPHORE_RANGE_CLEAR", "EventSemaphore"):
                continue
            if nw > worst.get(op, (0, None))[0]:
                worst[op] = (nw, inst.name)
    return worst


def _build_nc(T=T_FULL, C=32):
    import concourse.bass as bass
    import concourse.mybir as mybir
    import concourse.tile as tile

    dt = mybir.dt
    AF = mybir.ActivationFunctionType
    OP = mybir.AluOpType

    assert T % C == 0
    NCH = T // C
    FB = B_LOC * C  # free elements per (mt) psum group / per kt block

    nc = bass.Bass()

    xt = nc.declare_dram_parameter("xt", [128, T * B_LOC], dt.bfloat16, isOutput=False)
    whd = [
        nc.declare_dram_parameter(f"wh{l}", [128, 1024], dt.bfloat16, isOutput=False)
        for l in range(2)
    ]
    wx0d = nc.declare_dram_parameter("wx0", [128, 512], dt.bfloat16, isOutput=False)
    wx1d = nc.declare_dram_parameter("wx1", [128, 1024], dt.bfloat16, isOutput=False)
    biasd = [
        nc.declare_dram_parameter(f"bias{l}", [1, 512], dt.bfloat16, isOutput=False)
        for l in range(2)
    ]
    yd = nc.declare_dram_parameter("y", [128, T * 2 * B_LOC], dt.bfloat16, isOutput=True)
    hfd = nc.declare_dram_parameter("hfinal", [128, 4 * B_LOC], dt.bfloat16, isOutput=True)

    # PSUM layout is STEP-major: step s occupies ps[:, s*32:(s+1)*32] with
    # (group, batch) inside, so the per-step tanh reads a dense [128, 32].
    # Bulk x-proj matmuls write strided views, split per PSUM bank (a matmul
    # output must stay within one bank; a bank holds 16 steps).

    def emit_bulk(ps, wxt, nkt, rhs_kt, bias, absorber_src):
        """x-projection + bias into psum for one chunk.

        wxt: weights [128, nkt*512]; rhs_kt(kt) -> AP [128, C, 8] (or 2D)
        absorber_src: [1,1] ACT-written AP to absorb the psum-slot WAR tick.
        """
        if absorber_src is not None:
            nc.tensor.matmul(
                ps[0:1, 1:2], absorber_src, absorber_src,
                start=False, stop=False, skip_group_check=True,
            )
        psv = ps[:, :].rearrange("p (s g f) -> p g s f", g=4, f=B_LOC)
        for half in range(2):
            sl = slice(half * (C // 2), (half + 1) * (C // 2))
            for mt in range(4):
                for kt in range(nkt):
                    nc.tensor.matmul(
                        psv[:, mt, sl],
                        wxt[:, kt * 512 + mt * 128 : kt * 512 + (mt + 1) * 128],
                        rhs_kt(kt)[:, sl],
                        start=(mt == 0 and kt == 0),
                        stop=False,
                        skip_group_check=True,
                    )
            for mt in range(4):
                nc.tensor.matmul(
                    psv[:, mt, sl],
                    bias[0:1, mt * 128 : (mt + 1) * 128],
                    ones_t[0:1, half * (C // 2) * B_LOC : (half + 1) * (C // 2) * B_LOC],
                    start=False,
                    stop=False,
                    skip_group_check=True,
                )

    def start_chunk(L, ps, hc, tallc, use_dpb):
        L["ps"] = ps
        L["prev_hc"], L["hc"] = L.get("hc"), hc
        L["prev_tall"], L["tall"] = L.get("tall"), tallc
        if use_dpb:
            # Absorber: DVE observes current PE ticks so the first write into
            # the recycled h-chunk tile (WAR vs old PE readers) is covered.
            nc.vector.tensor_copy(L["dpb"], ps[0:1, 513:514])
            probe = L["dpb"]
        else:
            probe = None if L["prev_hc"] is None else L["prev_hc"][0:1, (C - 1) * 16 : (C - 1) * 16 + 1]
        if probe is not None:
            # Absorber: ACT observes recent DVE ticks (buffer WAR + psum bank
            # read serialization) via a single-wait op.
            nc.scalar.copy(L["sa"], probe)

    def emit_step(L, s, solo=False):
        ps, hc, tallc, wht = L["ps"], L["hc"], L["tall"], L["wht"]
        first = s == 0 and L["prev_hc"] is None
        hp = (
            L["prev_hc"][:, (C - 1) * 16 : C * 16]
            if s == 0 and L["prev_hc"] is not None
            else hc[:, (s - 1) * 16 : s * 16]
        )
        tall = tallc[:, s * 32 : (s + 1) * 32]
        if not first:
            # Absorber: 1x1 matmul reading tall(s-1) so PE observes the
            # act(s-1) tick; the real matmuls then only wait on DVE.
            # Writes a dead psum cell (step-0 region, already consumed).
            prev_tall = tallc[0:1, (s - 1) * 32 : (s - 1) * 32 + 1]
            nc.tensor.matmul(
                ps[0:1, 1:2], prev_tall, prev_tall,
                start=False, stop=False, skip_group_check=True,
            )
            for mt in range(4):
                for kt in range(2):
                    nc.tensor.matmul(
                        ps[:, s * 32 + mt * B_LOC : s * 32 + (mt + 1) * B_LOC],
                        wht[:, kt * 512 + mt * 128 : kt * 512 + (mt + 1) * 128],
                        hp[:, kt * B_LOC : (kt + 1) * B_LOC],
                        start=False,
                        stop=(s == C - 1 and kt == 1 and mt == 3),
                        skip_group_check=True,
                    )
        nc.scalar.activation(tall, ps[:, s * 32 : (s + 1) * 32], AF.Tanh)
        u = tmpp.tile([128, 16], dt.float32, tag=f"u{L['l']}", name="u")
        if first:
            # h_prev = 0: h_new = alpha*cand = (0.5*t_a+0.5)*cand
            nc.vector.scalar_tensor_tensor(
                u[:, :], tall[:, 16:32], 1.0, tall[:, 0:16], OP.add, OP.mult
            )
            nc.vector.tensor_scalar(hc[:, 0:16], u[:, :], 0.5, None, OP.mult)
            return
        if solo:
            # No other layer interleaved: the d-STT's RAW producer h(s-1) is
            # adjacent on DVE, so it must keep its self wait; absorb the act
            # tick with a 1-elem copy.
            nc.vector.tensor_copy(L["tpb"], tall[0:1, 0:1])
        d = tmpp.tile([128, 16], dt.float32, tag=f"d{L['l']}", name="d")
        # d = cand - h_prev
        nc.vector.scalar_tensor_tensor(
            d[:, :], tall[:, 0:16], 1.0, hp, OP.mult, OP.subtract
        )
        # u = (tanh_alpha + 1) * d
        nc.vector.scalar_tensor_tensor(
            u[:, :], tall[:, 16:32], 1.0, d[:, :], OP.add, OP.mult
        )
        # h_new = u * 0.5 + h_prev   (= alpha*cand + (1-alpha)*h)
        nc.vector.scalar_tensor_tensor(
            hc[:, s * 16 : (s + 1) * 16], u[:, :], 0.5, hp, OP.mult, OP.add
        )

    with tile.TileContext(nc) as tc:
        with (
            tc.tile_pool(name="const", bufs=1) as constp,
            tc.tile_pool(name="xin", bufs=1) as xp,
            tc.tile_pool(name="h0b", bufs=3) as h0p,
            tc.tile_pool(name="ysb", bufs=1) as ysbp,
            tc.tile_pool(name="tl", bufs=2) as tallp,
            tc.tile_pool(name="tmp", bufs=4) as tmpp,
            tc.tile_pool(name="sab", bufs=2) as sap,
            tc.tile_pool(name="ps0", bufs=2, space=bass.MemorySpace.PSUM) as ps0p,
            tc.tile_pool(name="ps1", bufs=2, space=bass.MemorySpace.PSUM) as ps1p,
        ):
            wh_t = [
                constp.tile([128, 1024], dt.bfloat16, tag=f"wh{l}", name=f"wh{l}")
                for l in range(2)
            ]
            wx0_t = constp.tile([128, 512], dt.bfloat16, tag="wx0")
            wx1_t = constp.tile([128, 1024], dt.bfloat16, tag="wx1")
            bias_t = [
                constp.tile([1, 512], dt.bfloat16, tag=f"b{l}", name=f"b{l}")
                for l in range(2)
            ]
            ones_t = constp.tile([1, FB], dt.bfloat16, tag="ones")

            for l in range(2):
                nc.sync.dma_start(wh_t[l][:, :], whd[l][:, :])
                nc.sync.dma_start(bias_t[l][:, :], biasd[l][:, :])
            nc.sync.dma_start(wx0_t[:, :], wx0d[:, :])
            nc.sync.dma_start(wx1_t[:, :], wx1d[:, :])
            nc.gpsimd.memset(ones_t[:, :], 1.0)

            y_sbuf = ysbp.tile([128, T * 16], dt.bfloat16, tag="ysb", name="ysb")
            sa0 = sap.tile([1, 1], dt.float32, tag="sa0", name="sa0")
            sa1 = sap.tile([1, 1], dt.float32, tag="sa1", name="sa1")
            dpb0 = sap.tile([1, 1], dt.float32, tag="dpb0", name="dpb0")
            tpb0 = sap.tile([1, 1], dt.float32, tag="tpb0", name="tpb0")
            tpb1 = sap.tile([1, 1], dt.float32, tag="tpb1", name="tpb1")
            L0 = {"l": 0, "wht": wh_t[0], "sa": sa0[0:1, 0:1], "dpb": dpb0[0:1, 0:1], "tpb": tpb0[0:1, 0:1]}
            L1 = {"l": 1, "wht": wh_t[1], "sa": sa1[0:1, 0:1], "dpb": None, "tpb": tpb1[0:1, 0:1]}

            def bulk0(k):
                xt_tile = xp.tile([128, FB], dt.bfloat16, tag=f"x{k % 4}", name="x")
                nc.sync.dma_start(xt_tile[:, :], xt[:, k * FB : (k + 1) * FB])
                ps0 = ps0p.tile([128, 4 * FB], dt.float32, tag="ps0", name="ps0")
                xv = xt_tile[:, :].rearrange("p (s f) -> p s f", f=B_LOC)
                ab = (
                    None
                    if L0.get("tall") is None
                    else L0["tall"][0:1, (C - 1) * 32 : (C - 1) * 32 + 1]
                )
                emit_bulk(ps0, wx0_t, 1, lambda kt: xv, bias_t[0], ab)
                return ps0

            def bulk1(k, h0c):
                ps1 = ps1p.tile([128, 4 * FB], dt.float32, tag="ps1", name="ps1")
                h0_3d = h0c[:, :].rearrange("p (s k f) -> p k s f", k=2, f=B_LOC)
                emit_bulk(ps1, wx1_t, 2, lambda kt: h0_3d[:, kt], bias_t[1], None)
                return ps1

            def l0_chunk_alloc(k):
                h0c = h0p.tile([128, C * 16], dt.bfloat16, tag="h0", name="h0")
                tl = tallp.tile([128, C * 32], dt.float32, tag="tl0", name="tl0")
                return h0c, tl

            # ---- software pipeline: l1 runs one chunk behind l0, their
            # serial chains interleaved step-by-step so the engines overlap
            # the two independent recurrences. ----
            ps0 = bulk0(0)
            h0c, tl0 = l0_chunk_alloc(0)
            start_chunk(L0, ps0, h0c, tl0, use_dpb=False)
            for s in range(C):
                emit_step(L0, s, solo=True)

            ydma = None
            for k in range(1, NCH):
                ps1 = bulk1(k - 1, L0["hc"])
                tl1 = tallp.tile([128, C * 32], dt.float32, tag="tl1", name="tl1")
                h1c = y_sbuf[:, (k - 1) * C * 16 : k * C * 16]
                start_chunk(L1, ps1, h1c, tl1, use_dpb=False)
                ps0 = bulk0(k)
                h0c, tl0 = l0_chunk_alloc(k)
                start_chunk(L0, ps0, h0c, tl0, use_dpb=True)
                for s in range(C):
                    emit_step(L1, s)
                    emit_step(L0, s)
                kdone = k - 1  # l1 chunk finished this iteration
                if (kdone + 1) % 8 == 0:
                    lo = (kdone // 8) * 8 * 16 * C
                    ydma = nc.sync.dma_start(
                        yd[:, lo : (kdone + 1) * 16 * C],
                        y_sbuf[:, lo : (kdone + 1) * 16 * C],
                    )

            # epilogue: last l1 chunk alone
            ps1 = bulk1(NCH - 1, L0["hc"])
            tl1 = tallp.tile([128, C * 32], dt.float32, tag="tl1", name="tl1")
            h1c = y_sbuf[:, (NCH - 1) * C * 16 : NCH * C * 16]
            start_chunk(L1, ps1, h1c, tl1, use_dpb=False)
            for s in range(C):
                emit_step(L1, s, solo=True)
            lo = ((NCH - 1) // 8) * 8 * 16 * C
            ydma = nc.sync.dma_start(
                yd[:, lo : NCH * 16 * C], y_sbuf[:, lo : NCH * 16 * C]
            )

            hf0 = sap.tile([128, 16], dt.bfloat16, tag="hf0", name="hf0")
            hf1 = sap.tile([128, 16], dt.bfloat16, tag="hf1", name="hf1")
            nc.vector.tensor_copy(hf0[:, :], L0["hc"][:, (C - 1) * 16 : C * 16])
            nc.vector.tensor_copy(hf1[:, :], L1["hc"][:, (C - 1) * 16 : C * 16])
            hfdma0 = nc.sync.dma_start(hfd[:, 0:16], hf0[:, :])
            hfdma1 = nc.sync.dma_start(hfd[:, 16:32], hf1[:, :])

            # ---- ring-drain absorber chain ----
            # The kernel-tail Drain may carry only one sync wait. 8 chained
            # 4-byte DRAM->DRAM DMAs walk all 8 HW-DGE rings (round-robin
            # assignment); ring FIFO + the RAW chain make the last absorber's
            # completion imply every earlier DMA completed, so the drain ends
            # up waiting on a single semaphore.
            import bass_rust as _br

            scratch = nc.dram_tensor("ringchain", [1, 16], dt.float32)
            prev_ab = None
            for i in range(8):
                ab = nc.sync.dma_start(
                    scratch[0:1, i + 1 : i + 2], scratch[0:1, i : i + 1]
                )
                if prev_ab is None:
                    for o in (ydma, hfdma0, hfdma1):
                        _br.add_dep_helper(
                            ab.ins, o.ins, sync=False, reason="ring-drain order"
                        )
                else:
                    _br.add_dep_helper(
                        ab.ins, prev_ab.ins, sync=True, reason="ring-drain chain"
                    )
                prev_ab = ab

    _drop_self_waits(nc)
    return nc


def _make_inmaps(inputs, T=T_FULL):
    import ml_dtypes

    bf = ml_dtypes.bfloat16
    x = np.asarray(inputs["input"], dtype=np.float32)[:, :T, :]

    wh, bias, wx = [], [], []
    for l in range(2):
        Whcat = np.concatenate(
            [np.asarray(inputs[f"W_hh_{l}"]), 0.5 * np.asarray(inputs[f"W_ah_{l}"])],
            axis=1,
        )
        b = np.concatenate(
            [np.asarray(inputs[f"b_h_{l}"]), 0.5 * np.asarray(inputs[f"b_a_{l}"])]
        ).reshape(1, 512)
        wh.append(np.concatenate([Whcat[:128], Whcat[128:]], axis=1).astype(bf))
        bias.append(b.astype(bf))
        Wxcat = np.concatenate(
            [np.asarray(inputs[f"W_ih_{l}"]), 0.5 * np.asarray(inputs[f"W_ax_{l}"])],
            axis=1,
        )
        if l == 0:
            wx.append(Wxcat.astype(bf))  # [128, 512]
        else:
            wx.append(
                np.concatenate([Wxcat[:128], Wxcat[128:]], axis=1).astype(bf)
            )  # [128, 1024]

    xt_full = np.ascontiguousarray(x.transpose(2, 1, 0)).astype(bf)  # [128, T, 64]
    in_maps = []
    for i in range(NCORES):
        shard = np.ascontiguousarray(
            xt_full[:, :, i * B_LOC : (i + 1) * B_LOC]
        ).reshape(128, T * B_LOC)
        in_maps.append(
            {
                "xt": shard,
                "wh0": wh[0],
                "wh1": wh[1],
                "wx0": wx[0],
                "wx1": wx[1],
                "bias0": bias[0],
                "bias1": bias[1],
            }
        )
    return in_maps


def _postprocess(results, T=T_FULL):
    ys, hfs = [], []
    for i in range(NCORES):
        a = np.asarray(results[i]["y"]).astype(np.float32).reshape(128, T, 2, B_LOC)
        ys.append(np.ascontiguousarray(a.transpose(3, 1, 2, 0)).reshape(B_LOC, T, 256))
        hf = np.asarray(results[i]["hfinal"]).astype(np.float32).reshape(128, 2, 2, B_LOC)
        hfs.append(np.ascontiguousarray(hf.transpose(1, 3, 2, 0)).reshape(2, B_LOC, 256))
    y = np.concatenate(ys, axis=0)
    hf = np.concatenate(hfs, axis=1)
    return y, hf


def kernel(**inputs):
    from concourse.bass_utils import run_bass_kernel_spmd

    nc = _build_nc()
    in_maps = _make_inmaps(inputs)
    res = run_bass_kernel_spmd(nc, in_maps, list(range(NCORES)))
    y, hf = _postprocess(res.results)
    return y, hf


# revision 29
# speedup vs baseline: 1.0033x; 1.0033x over previous
"""AlphatRNN Trainium2 kernel (8 NeuronCores, data-parallel over batch).

Problem: 2-layer alpha-RNN, B=64, T=2048, I=128, H=256.
  per step t, layer l:
    cand  = tanh(cur @ Wih + h_l @ Whh + bh)
    alpha = sigmoid(cur @ Wax + h_l @ Wah + ba)
    h_l   = alpha * cand + (1 - alpha) * h_l ;  cur = h_l
  outputs = h_1 over time [B, T, H];  h_final [2, B, H]

Design notes:
  - Batch sharded 8 ways (8 rows/core); both layers per core; T is serial.
  - Everything lives transposed on chip: h as [H partitions, (step, batch)].
  - sigmoid(z) = 0.5*tanh(z/2)+0.5 folded into weights => ONE tanh per
    layer-step over [128, 32] covering cand|alpha pre-activations.
  - x-projections computed in bulk per chunk of C steps directly into PSUM;
    per-step recurrent matmuls accumulate into the same PSUM regions
    (has_written bits stay set), so no separate add op is needed.
  - h state kept in bf16 (validated: rel_l2 ~5e-3 vs f32 reference).
  - blend uses fused scalar_tensor_tensor ops: 3 VectorE ops per layer-step.
  - Hardware allows ONE semaphore wait per instruction. Tile emits a wait per
    unobserved dep proc, including redundant same-engine ones. We (a) drop
    same-engine waits post-schedule (engines execute/complete in FIFO order),
    and (b) insert tiny "absorber" ops so each cross-engine tick is observed
    by an instruction that needs no other wait.
"""

import numpy as np

B, T_FULL, I, H = 64, 2048, 128, 256
NCORES = 8
B_LOC = B // NCORES  # 8


def _drop_self_waits(nc):
    """Remove waits on an engine's own semaphore: engines issue and complete
    instructions in order, so same-engine deps are satisfied by program order.
    Necessary because most instruction encodings have one sync-wait slot."""
    import bass_rust

    prefix = {
        "EngineType.PE": "PE_",
        "EngineType.DVE": "DVE_",
        "EngineType.Activation": "Activation_",
    }
    # Identify the ringchain absorbers (the last 8 DMACopies): verify they
    # cover all 8 rings and find the last one's ring semaphore.
    dmas = []
    for blk in nc.m.functions[0].blocks:
        for inst in blk.instructions:
            if inst.concise_opcode() == "DMACopy":
                dmas.append(inst)
    last_ring_sem = None
    if len(dmas) >= 8:
        tailsems = [
            d.sync_info.on_update[0].ant_name
            for d in dmas[-8:]
            if d.sync_info and d.sync_info.on_update
        ]
        if len(set(tailsems)) == 8:
            last_ring_sem = tailsems[-1]
    name2inst = {}
    engpos = {}
    engcnt = {}
    for blk in nc.m.functions[0].blocks:
        for inst in blk.instructions:
            name2inst[inst.name] = inst
            e = str(inst.engine)
            engpos[inst.name] = engcnt.get(e, 0)
            engcnt[e] = engcnt.get(e, 0) + 1
    observed = {}
    n_drop = 0
    for blk in nc.m.functions[0].blocks:
        for inst in blk.instructions:
            si = inst.sync_info
            if not si or not si.on_wait:
                continue
            if inst.concise_opcode() == "Drain":
                # Engine-stream completion is implied by the all-engine
                # barrier that follows; DMA-ring completion is implied by the
                # ringchain absorber chain: the last absorber's ring semaphore
                # transitively covers every ring (chain RAW waits + ring
                # FIFO). So the drain needs exactly that one wait.
                if last_ring_sem is not None and len(si.on_wait) > 1:
                    kept = [w for w in si.on_wait if w.ant_name == last_ring_sem]
                    assert len(kept) == 1, (
                        f"ringchain drain fix: expected 1 wait on "
                        f"{last_ring_sem}, got {[(w.ant_name, w.wait_value) for w in si.on_wait]}"
                    )
                    n_drop += len(si.on_wait) - len(kept)
                    inst.sync_info = bass_rust.SyncInfo(
                        on_wait=kept, on_update=list(si.on_update)
                    )
                continue
            if inst.concise_opcode() == "DMACopy":
                # A DMA ring executes its descriptors serially, so a wait on a
                # semaphore this DMA itself updates (= its own ring's sem) is
                # satisfied by ring order.
                own = {u.ant_name for u in si.on_update}
                kept = [w for w in si.on_wait if w.ant_name not in own]
                if len(kept) == 2:
                    # [engine-wait, DMAHW-wait] = [slot readers, slot's prior
                    # writer]. Readers waited on the prior write themselves,
                    # so the engine wait transitively covers the DMAHW one.
                    eng_w = [w for w in kept if not w.ant_name.startswith("DMAHW")]
                    dma_w = [w for w in kept if w.ant_name.startswith("DMAHW")]
                    if len(eng_w) == 1 and len(dma_w) == 1:
                        kept = eng_w
                if len(kept) != len(si.on_wait):
                    n_drop += len(si.on_wait) - len(kept)
                    inst.sync_info = bass_rust.SyncInfo(
                        on_wait=kept, on_update=list(si.on_update)
                    )
                continue
            if inst.concise_opcode() in (
                "EventSemaphore",
                "EVENT_SEMAPHORE_RANGE_CLEAR",
            ):
                continue
            eng = str(inst.engine)
            pref = prefix.get(eng)
            if pref is None:
                continue
            obs = observed.setdefault(eng, {})
            # Same-engine RAW dep? Then the self-sem wait is load-bearing:
            # the engine pipeline lets op N+1 read inputs before op N's
            # writes drain (measured ~100ns overlap on DVE), so only a
            # semaphore wait (sem incs at completion) makes the write
            # visible. WAR/WAW self deps are safe: reads happen in early
            # pipe stages, writes commit in order at the end.
            self_raw = False
            try:
                my_ins = {
                    a.memref for a in inst.ins if hasattr(a, "memref")
                }
            except Exception:
                my_ins = set()
            try:
                depnames = list(inst.sync_dependency_names())
            except Exception:
                depnames = []
            for dn in depnames:
                dep = name2inst.get(dn)
                if dep is None or str(dep.engine) != eng:
                    continue
                # A same-engine RAW hazard only bites within the engine's
                # pipeline depth (~1 op of overlap observed); a producer >=3
                # slots back has fully drained by issue order.
                if engpos[inst.name] - engpos[dn] >= 3:
                    continue
                try:
                    douts = {a.memref for a in dep.outs if hasattr(a, "memref")}
                except Exception:
                    douts = set()
                if douts & my_ins:
                    self_raw = True
                    break
            kept = []
            for w in si.on_wait:
                nm = w.ant_name or ""
                if obs.get(nm, -1) >= w.wait_value:
                    continue  # already observed by this engine: redundant
                if nm.startswith(pref) and not self_raw:
                    continue  # self-sem wait guarding only WAR/WAW: safe
                kept.append(w)
            assert len(kept) <= 1, (
                f"{inst.name} {inst.concise_opcode()} on {eng} still needs "
                f"{[(w.ant_name, w.wait_value) for w in kept]} "
                f"(self_raw={self_raw}) - add an absorber op"
            )
            for w in kept:
                obs[w.ant_name] = max(obs.get(w.ant_name, -1), w.wait_value)
            if len(kept) != len(si.on_wait):
                n_drop += len(si.on_wait) - len(kept)
                inst.sync_info = bass_rust.SyncInfo(
                    on_wait=kept, on_update=list(si.on_update)
                )
    return n_drop


def _max_waits(nc):
    worst = {}
    for blk in nc.m.functions[0].blocks:
        for inst in blk.instructions:
            si = inst.sync_info
            nw = len(si.on_wait) if si else 0
            op = inst.concise_opcode()
            if op in ("Drain", "EVENT_SEMAPHORE_RANGE_CLEAR", "EventSemaphore"):
                continue
            if nw > worst.get(op, (0, None))[0]:
                worst[op] = (nw, inst.name)
    return worst


def _build_nc(T=T_FULL, C=32):
    import concourse.bass as bass
    import concourse.mybir as mybir
    import concourse.tile as tile

    dt = mybir.dt
    AF = mybir.ActivationFunctionType
    OP = mybir.AluOpType

    assert T % C == 0
    NCH = T // C
    FB = B_LOC * C  # free elements per (mt) psum group / per kt block

    nc = bass.Bass()

    xt = nc.declare_dram_parameter("xt", [128, T * B_LOC], dt.bfloat16, isOutput=False)
    whd = [
        nc.declare_dram_parameter(f"wh{l}", [128, 1024], dt.bfloat16, isOutput=False)
        for l in range(2)
    ]
    wx0d = nc.declare_dram_parameter("wx0", [128, 512], dt.bfloat16, isOutput=False)
    wx1d = nc.declare_dram_parameter("wx1", [128, 1024], dt.bfloat16, isOutput=False)
    biasd = [
        nc.declare_dram_parameter(f"bias{l}", [1, 512], dt.bfloat16, isOutput=False)
        for l in range(2)
    ]
    yd = nc.declare_dram_parameter("y", [128, T * 2 * B_LOC], dt.bfloat16, isOutput=True)
    hfd = nc.declare_dram_parameter("hfinal", [128, 4 * B_LOC], dt.bfloat16, isOutput=True)

    # PSUM layout is STEP-major: step s occupies ps[:, s*32:(s+1)*32] with
    # (group, batch) inside, so the per-step tanh reads a dense [128, 32].
    # Bulk x-proj matmuls write strided views, split per PSUM bank (a matmul
    # output must stay within one bank; a bank holds 16 steps).

    def emit_bulk(ps, wxt, nkt, rhs_kt, bias, absorber_src):
        """x-projection + bias into psum for one chunk.

        wxt: weights [128, nkt*512]; rhs_kt(kt) -> AP [128, C, 8] (or 2D)
        absorber_src: [1,1] ACT-written AP to absorb the psum-slot WAR tick.
        """
        if absorber_src is not None:
            nc.tensor.matmul(
                ps[0:1, 1:2], absorber_src, absorber_src,
                start=False, stop=False, skip_group_check=True,
            )
        psv = ps[:, :].rearrange("p (s g f) -> p g s f", g=4, f=B_LOC)
        for half in range(2):
            sl = slice(half * (C // 2), (half + 1) * (C // 2))
            for mt in range(4):
                for kt in range(nkt):
                    nc.tensor.matmul(
                        psv[:, mt, sl],
                        wxt[:, kt * 512 + mt * 128 : kt * 512 + (mt + 1) * 128],
                        rhs_kt(kt)[:, sl],
                        start=(mt == 0 and kt == 0),
                        stop=False,
                        skip_group_check=True,
                    )
            for mt in range(4):
                nc.tensor.matmul(
                    psv[:, mt, sl],
                    bias[0:1, mt * 128 : (mt + 1) * 128],
                    ones_t[0:1, half * (C // 2) * B_LOC : (half + 1) * (C // 2) * B_LOC],
                    start=False,
                    stop=False,
                    skip_group_check=True,
                )

    def start_chunk(L, ps, hc, tallc, use_dpb):
        L["ps"] = ps
        L["prev_hc"], L["hc"] = L.get("hc"), hc
        L["prev_tall"], L["tall"] = L.get("tall"), tallc
        if use_dpb:
            # Absorber: DVE observes current PE ticks so the first write into
            # the recycled h-chunk tile (WAR vs old PE readers) is covered.
            nc.vector.tensor_copy(L["dpb"], ps[0:1, 513:514])
            probe = L["dpb"]
        else:
            probe = None if L["prev_hc"] is None else L["prev_hc"][0:1, (C - 1) * 16 : (C - 1) * 16 + 1]
        if probe is not None:
            # Absorber: ACT observes recent DVE ticks (buffer WAR + psum bank
            # read serialization) via a single-wait op.
            nc.scalar.copy(L["sa"], probe)

    def emit_step(L, s, solo=False):
        ps, hc, tallc, wht = L["ps"], L["hc"], L["tall"], L["wht"]
        first = s == 0 and L["prev_hc"] is None
        hp = (
            L["prev_hc"][:, (C - 1) * 16 : C * 16]
            if s == 0 and L["prev_hc"] is not None
            else hc[:, (s - 1) * 16 : s * 16]
        )
        tall = tallc[:, s * 32 : (s + 1) * 32]
        if not first:
            # Absorber: 1x1 matmul reading tall(s-1) so PE observes the
            # act(s-1) tick; the real matmuls then only wait on DVE.
            # Writes a dead psum cell (step-0 region, already consumed).
            prev_tall = tallc[0:1, (s - 1) * 32 : (s - 1) * 32 + 1]
            nc.tensor.matmul(
                ps[0:1, 1:2], prev_tall, prev_tall,
                start=False, stop=False, skip_group_check=True,
            )
            for mt in range(4):
                for kt in range(2):
                    nc.tensor.matmul(
                        ps[:, s * 32 + mt * B_LOC : s * 32 + (mt + 1) * B_LOC],
                        wht[:, kt * 512 + mt * 128 : kt * 512 + (mt + 1) * 128],
                        hp[:, kt * B_LOC : (kt + 1) * B_LOC],
                        start=False,
                        stop=(s == C - 1 and kt == 1 and mt == 3),
                        skip_group_check=True,
                    )
        nc.scalar.activation(tall, ps[:, s * 32 : (s + 1) * 32], AF.Tanh)
        u = tmpp.tile([128, 16], dt.float32, tag=f"u{L['l']}", name="u")
        if first:
            # h_prev = 0: h_new = alpha*cand = (0.5*t_a+0.5)*cand
            nc.vector.scalar_tensor_tensor(
                u[:, :], tall[:, 16:32], 1.0, tall[:, 0:16], OP.add, OP.mult
            )
            nc.vector.tensor_scalar(hc[:, 0:16], u[:, :], 0.5, None, OP.mult)
            return
        if solo:
            # No other layer interleaved: the d-STT's RAW producer h(s-1) is
            # adjacent on DVE, so it must keep its self wait; absorb the act
            # tick with a 1-elem copy.
            nc.vector.tensor_copy(L["tpb"], tall[0:1, 0:1])
        d = tmpp.tile([128, 16], dt.float32, tag=f"d{L['l']}", name="d")
        # d = cand - h_prev
        nc.vector.scalar_tensor_tensor(
            d[:, :], tall[:, 0:16], 1.0, hp, OP.mult, OP.subtract
        )
        # u = (tanh_alpha + 1) * d
        nc.vector.scalar_tensor_tensor(
            u[:, :], tall[:, 16:32], 1.0, d[:, :], OP.add, OP.mult
        )
        # h_new = u * 0.5 + h_prev   (= alpha*cand + (1-alpha)*h)
        nc.vector.scalar_tensor_tensor(
            hc[:, s * 16 : (s + 1) * 16], u[:, :], 0.5, hp, OP.mult, OP.add
        )

    with tile.TileContext(nc) as tc:
        with (
            tc.tile_pool(name="const", bufs=1) as constp,
            tc.tile_pool(name="xin", bufs=1) as xp,
            tc.tile_pool(name="h0b", bufs=3) as h0p,
            tc.tile_pool(name="ysb", bufs=1) as ysbp,
            tc.tile_pool(name="tl", bufs=2) as tallp,
            tc.tile_pool(name="tmp", bufs=4) as tmpp,
            tc.tile_pool(name="sab", bufs=2) as sap,
            tc.tile_pool(name="ps0", bufs=2, space=bass.MemorySpace.PSUM) as ps0p,
            tc.tile_pool(name="ps1", bufs=2, space=bass.MemorySpace.PSUM) as ps1p,
        ):
            wh_t = [
                constp.tile([128, 1024], dt.bfloat16, tag=f"wh{l}", name=f"wh{l}")
                for l in range(2)
            ]
            wx0_t = constp.tile([128, 512], dt.bfloat16, tag="wx0")
            wx1_t = constp.tile([128, 1024], dt.bfloat16, tag="wx1")
            bias_t = [
                constp.tile([1, 512], dt.bfloat16, tag=f"b{l}", name=f"b{l}")
                for l in range(2)
            ]
            ones_t = constp.tile([1, FB], dt.bfloat16, tag="ones")

            for l in range(2):
                nc.sync.dma_start(wh_t[l][:, :], whd[l][:, :])
                nc.sync.dma_start(bias_t[l][:, :], biasd[l][:, :])
            nc.sync.dma_start(wx0_t[:, :], wx0d[:, :])
            nc.sync.dma_start(wx1_t[:, :], wx1d[:, :])
            nc.gpsimd.memset(ones_t[:, :], 1.0)

            y_sbuf = ysbp.tile([128, T * 16], dt.bfloat16, tag="ysb", name="ysb")
            sa0 = sap.tile([1, 1], dt.float32, tag="sa0", name="sa0")
            sa1 = sap.tile([1, 1], dt.float32, tag="sa1", name="sa1")
            dpb0 = sap.tile([1, 1], dt.float32, tag="dpb0", name="dpb0")
            tpb0 = sap.tile([1, 1], dt.float32, tag="tpb0", name="tpb0")
            tpb1 = sap.tile([1, 1], dt.float32, tag="tpb1", name="tpb1")
            L0 = {"l": 0, "wht": wh_t[0], "sa": sa0[0:1, 0:1], "dpb": dpb0[0:1, 0:1], "tpb": tpb0[0:1, 0:1]}
            L1 = {"l": 1, "wht": wh_t[1], "sa": sa1[0:1, 0:1], "dpb": None, "tpb": tpb1[0:1, 0:1]}

            def bulk0(k):
                xt_tile = xp.tile([128, FB], dt.bfloat16, tag=f"x{k % 4}", name="x")
                nc.sync.dma_start(xt_tile[:, :], xt[:, k * FB : (k + 1) * FB])
                ps0 = ps0p.tile([128, 4 * FB], dt.float32, tag="ps0", name="ps0")
                xv = xt_tile[:, :].rearrange("p (s f) -> p s f", f=B_LOC)
                ab = (
                    None
                    if L0.get("tall") is None
                    else L0["tall"][0:1, (C - 1) * 32 : (C - 1) * 32 + 1]
                )
                emit_bulk(ps0, wx0_t, 1, lambda kt: xv, bias_t[0], ab)
                return ps0

            def bulk1(k, h0c):
                ps1 = ps1p.tile([128, 4 * FB], dt.float32, tag="ps1", name="ps1")
                h0_3d = h0c[:, :].rearrange("p (s k f) -> p k s f", k=2, f=B_LOC)
                emit_bulk(ps1, wx1_t, 2, lambda kt: h0_3d[:, kt], bias_t[1], None)
                return ps1

            def l0_chunk_alloc(k):
                h0c = h0p.tile([128, C * 16], dt.bfloat16, tag="h0", name="h0")
                tl = tallp.tile([128, C * 32], dt.bfloat16, tag="tl0", name="tl0")
                return h0c, tl

            # ---- software pipeline: l1 runs one chunk behind l0, their
            # serial chains interleaved step-by-step so the engines overlap
            # the two independent recurrences. ----
            ps0 = bulk0(0)
            h0c, tl0 = l0_chunk_alloc(0)
            start_chunk(L0, ps0, h0c, tl0, use_dpb=False)
            for s in range(C):
                emit_step(L0, s, solo=True)

            ydma = None
            for k in range(1, NCH):
                ps1 = bulk1(k - 1, L0["hc"])
                tl1 = tallp.tile([128, C * 32], dt.bfloat16, tag="tl1", name="tl1")
                h1c = y_sbuf[:, (k - 1) * C * 16 : k * C * 16]
                start_chunk(L1, ps1, h1c, tl1, use_dpb=False)
                ps0 = bulk0(k)
                h0c, tl0 = l0_chunk_alloc(k)
                start_chunk(L0, ps0, h0c, tl0, use_dpb=True)
                for s in range(C):
                    emit_step(L1, s)
                    emit_step(L0, s)
                kdone = k - 1  # l1 chunk finished this iteration
                if (kdone + 1) % 8 == 0:
                    lo = (kdone // 8) * 8 * 16 * C
                    ydma = nc.sync.dma_start(
                        yd[:, lo : (kdone + 1) * 16 * C],
                        y_sbuf[:, lo : (kdone + 1) * 16 * C],
                    )

            # epilogue: last l1 chunk alone
            ps1 = bulk1(NCH - 1, L0["hc"])
            tl1 = tallp.tile([128, C * 32], dt.bfloat16, tag="tl1", name="tl1")
            h1c = y_sbuf[:, (NCH - 1) * C * 16 : NCH * C * 16]
            start_chunk(L1, ps1, h1c, tl1, use_dpb=False)
            for s in range(C):
                emit_step(L1, s, solo=True)
            lo = ((NCH - 1) // 8) * 8 * 16 * C
            ydma = nc.sync.dma_start(
                yd[:, lo : NCH * 16 * C], y_sbuf[:, lo : NCH * 16 * C]
            )

            hf0 = sap.tile([128, 16], dt.bfloat16, tag="hf0", name="hf0")
            hf1 = sap.tile([128, 16], dt.bfloat16, tag="hf1", name="hf1")
            nc.vector.tensor_copy(hf0[:, :], L0["hc"][:, (C - 1) * 16 : C * 16])
            nc.vector.tensor_copy(hf1[:, :], L1["hc"][:, (C - 1) * 16 : C * 16])
            hfdma0 = nc.sync.dma_start(hfd[:, 0:16], hf0[:, :])
            hfdma1 = nc.sync.dma_start(hfd[:, 16:32], hf1[:, :])

            # ---- ring-drain absorber chain ----
            # The kernel-tail Drain may carry only one sync wait. 8 chained
            # 4-byte DRAM->DRAM DMAs walk all 8 HW-DGE rings (round-robin
            # assignment); ring FIFO + the RAW chain make the last absorber's
            # completion imply every earlier DMA completed, so the drain ends
            # up waiting on a single semaphore.
            import bass_rust as _br

            scratch = nc.dram_tensor("ringchain", [1, 16], dt.float32)
            prev_ab = None
            for i in range(8):
                ab = nc.sync.dma_start(
                    scratch[0:1, i + 1 : i + 2], scratch[0:1, i : i + 1]
                )
                if prev_ab is None:
                    for o in (ydma, hfdma0, hfdma1):
                        _br.add_dep_helper(
                            ab.ins, o.ins, sync=False, reason="ring-drain order"
                        )
                else:
                    _br.add_dep_helper(
                        ab.ins, prev_ab.ins, sync=True, reason="ring-drain chain"
                    )
                prev_ab = ab

    _drop_self_waits(nc)
    return nc


def _make_inmaps(inputs, T=T_FULL):
    import ml_dtypes

    bf = ml_dtypes.bfloat16
    x = np.asarray(inputs["input"], dtype=np.float32)[:, :T, :]

    wh, bias, wx = [], [], []
    for l in range(2):
        Whcat = np.concatenate(
            [np.asarray(inputs[f"W_hh_{l}"]), 0.5 * np.asarray(inputs[f"W_ah_{l}"])],
            axis=1,
        )
        b = np.concatenate(
            [np.asarray(inputs[f"b_h_{l}"]), 0.5 * np.asarray(inputs[f"b_a_{l}"])]
        ).reshape(1, 512)
        wh.append(np.concatenate([Whcat[:128], Whcat[128:]], axis=1).astype(bf))
        bias.append(b.astype(bf))
        Wxcat = np.concatenate(
            [np.asarray(inputs[f"W_ih_{l}"]), 0.5 * np.asarray(inputs[f"W_ax_{l}"])],
            axis=1,
        )
        if l == 0:
            wx.append(Wxcat.astype(bf))  # [128, 512]
        else:
            wx.append(
                np.concatenate([Wxcat[:128], Wxcat[128:]], axis=1).astype(bf)
            )  # [128, 1024]

    xt_full = np.ascontiguousarray(x.transpose(2, 1, 0)).astype(bf)  # [128, T, 64]
    in_maps = []
    for i in range(NCORES):
        shard = np.ascontiguousarray(
            xt_full[:, :, i * B_LOC : (i + 1) * B_LOC]
        ).reshape(128, T * B_LOC)
        in_maps.append(
            {
                "xt": shard,
                "wh0": wh[0],
                "wh1": wh[1],
                "wx0": wx[0],
                "wx1": wx[1],
                "bias0": bias[0],
                "bias1": bias[1],
            }
        )
    return in_maps


def _postprocess(results, T=T_FULL):
    ys, hfs = [], []
    for i in range(NCORES):
        a = np.asarray(results[i]["y"]).astype(np.float32).reshape(128, T, 2, B_LOC)
        ys.append(np.ascontiguousarray(a.transpose(3, 1, 2, 0)).reshape(B_LOC, T, 256))
        hf = np.asarray(results[i]["hfinal"]).astype(np.float32).reshape(128, 2, 2, B_LOC)
        hfs.append(np.ascontiguousarray(hf.transpose(1, 3, 2, 0)).reshape(2, B_LOC, 256))
    y = np.concatenate(ys, axis=0)
    hf = np.concatenate(hfs, axis=1)
    return y, hf


def kernel(**inputs):
    from concourse.bass_utils import run_bass_kernel_spmd

    nc = _build_nc()
    in_maps = _make_inmaps(inputs)
    res = run_bass_kernel_spmd(nc, in_maps, list(range(NCORES)))
    y, hf = _postprocess(res.results)
    return y, hf
